# revision 38
# baseline (speedup 1.0000x reference)
"""Multi-Head Latent Attention (MLA) forward on 8 Trainium2 NeuronCores.

Contract: kernel(**inputs) takes the FULL unsharded inputs (numpy) and
returns the FULL [1, 4096, 2048] float32 output.

Sharding (hardcoded):
  - Tensor-parallel over heads: 2 heads per core (up-proj weights
    column-sharded, W_out row-sharded; partial outputs summed on host).
  - Down-projections (x @ W_dkv, x @ W_dq) are sharded over the sequence
    (512 rows per core) and AllGathered as fp8 hi+lo pairs.

fp8 strategy (e4m3, all tensors pre-scaled by power-of-2 so rms ~10):
  - down-proj: x and W_d both hi+lo fp8, 3-term DoubleRow (drop lo*lo)
    -> bf16-grade c at 0.75x bf16 PE cost.
  - c is evacuated as hi+lo fp8 and AllGathered packed ([2, C, LLOC]).
  - K/Q up-proj: single-fp8 (c_hi @ w_hi) DoubleRow -> 0.25x PE cost
    (softmax forgives the ~8% noise; validated 6.8e-3 rel err).
  - V up-proj: 3-term hi/lo DoubleRow (V accuracy matters) -> 0.75x.
  - scores: q/k stored fp8 [64p, 2(rope/base), L] per head (head h at
    partitions 64h..64h+64); DoubleRow contracts rope+base -> 0.5x.
  - PV + softmax: bf16 (fp8 there fails the accuracy budget).
  - out-proj: ctx split hi/lo on device (Pool engine), W_out hi/lo on
    host, 3-term DoubleRow -> 0.75x.
  - denominators: exact; per-block either DVE pair/quad tree (small
    blocks) or a bf16 ones-matmul column-sum on PE (large blocks,
    ONES_BLOCKS) -- PE reductions are ~3x cheaper than DVE here.

Device layout notes:
  - All activations transposed ([dim, L]); no on-device transposes.
  - RoPE: rope rows of both heads packed at psum A ([r_h0;r_h1]), base at
    psum B; the half-swap is a THIRD projection psC whose weight columns
    are permuted on the host (costs 2 fp8 matmuls/window, removes the
    Act-copy + perm-matmul chain); the psum->k8/q8 evac scale F_K folds
    into the host cos/sin tables (SQ chosen so F_Q == F_K).
  - Attention in S^T = [Lk, Lq] orientation, causally-restricted column
    tails on diagonal chunks; exp descale (1/(SQ*SK)) folds into the Act
    activation scale; SCTX folds into the V evac so normalize needs no
    extra scaling.
  - Scores are small (|s| < ~2), so exp() runs without max-subtraction.
"""

import sys

for _p in ("/opt/trn_rl_repo", "/opt/pypackages"):
    if _p not in sys.path:
        sys.path.insert(0, _p)

import math
import numpy as np
import ml_dtypes

import concourse.bacc as bacc
import concourse.mybir as mybir
import concourse.tile as tile
from concourse.bass_isa import ReduceOp as _ReduceOp
from concourse.bass_utils import run_bass_kernel_spmd

# Problem constants
L = 4096
D = 2048
C = 512
H = 16
HD = 128          # head dim
ROPE = 64
HALF = ROPE // 2  # 32
SPLIT = HD - ROPE # 64
N_CORES = 8
HPC = H // N_CORES   # heads per core = 2
LLOC = L // N_CORES  # 512 (down-proj shard)
BQ = 512             # Lq block
NB = L // BQ         # 8
NKC = L // 128       # 32 Lk chunks
DKC = D // 128       # 16
CKC = C // 128       # 4
ROPE_BASE = 10000.0

BF16 = mybir.dt.bfloat16
F32 = mybir.dt.float32
F8 = mybir.dt.float8e4
E4 = ml_dtypes.float8_e4m3
DR = mybir.MatmulPerfMode.DoubleRow

# fp8 power-of-2 scales (chosen so each quantized tensor has rms ~5-15,
# safely inside e4m3's [2^-6, 240] normal range)
SX, SWD, SC = 16.0, 256.0, 16.0
SWK, SWQ, SK, SQ = 256.0, 4096.0, 32.0, 512.0
SWV, SCTX, SWO = 256.0, 16.0, 256.0
F_K = SK / (SC * SWK)       # K up psum -> k8           (1/128)
F_Q = SQ / (SC * SWQ)       # Q up psum -> q8           (1/128, == F_K)
assert F_K == F_Q           # shared folded cc/ss tables require this
F_CEV = SC / (SX * SWD)     # down-proj psum -> c8      (1/256)
F_V = SCTX / (SC * SWV)     # V up psum -> vN (bf16)    (1/256)
F_O = 1.0 / (SCTX * SWO)    # out psum -> OUT (bf16)    (1/4096)
F_EXP = 1.0 / (SQ * SK)     # scores psum descale in exp (1/16384)

# blocks whose softmax denominator is computed by PE ones-matmul instead
# of the DVE pair/quad tree (engine balancing)
ONES_BLOCKS = (6, 7)

_CACHE = {}

# Ablation flags for subtractive profiling (timing only; output garbage when set)
OPTS = {
    "no_attn": False,
    "no_outproj": False,
    "no_ag": False,
    "no_denom": False,
    "no_upproj": False,
}


def _build_program(reps=1):
    nc = bacc.Bacc("TRN2", target_bir_lowering=False, debug=False, num_devices=N_CORES)

    xT8 = nc.dram_tensor("xT8", [2, D, LLOC], F8, kind="ExternalInput")
    wdkv8 = nc.dram_tensor("wdkv8", [2, D, C], F8, kind="ExternalInput")
    wdq8 = nc.dram_tensor("wdq8", [2, D, C], F8, kind="ExternalInput")
    # cols: [rope(128) | base(128) | rope-halfswapped(128)]
    wk8 = nc.dram_tensor("wk8", [C, 3 * HPC * ROPE], F8, kind="ExternalInput")
    wq8 = nc.dram_tensor("wq8", [C, 3 * HPC * ROPE], F8, kind="ExternalInput")
    wv8 = nc.dram_tensor("wv8", [2, C, HPC * HD], F8, kind="ExternalInput")
    wo8 = nc.dram_tensor("wo8", [2, HPC * HD, D], F8, kind="ExternalInput")
    CCd = nc.dram_tensor("CC", [128, L], BF16, kind="ExternalInput")
    SSd = nc.dram_tensor("SS", [128, L], BF16, kind="ExternalInput")
    CMd = nc.dram_tensor("CM", [4, 128, BQ], BF16, kind="ExternalInput")
    ONESd = nc.dram_tensor("ONES", [128, 1], BF16, kind="ExternalInput")
    OUT = nc.dram_tensor("OUT", [L, D], BF16, kind="ExternalOutput")

    agin0 = nc.dram_tensor("agin0", [2, C, LLOC], F8)
    agin1 = nc.dram_tensor("agin1", [2, C, LLOC], F8)
    agout0 = nc.dram_tensor("agout0", [N_CORES, 2, C, LLOC], F8, addr_space="Shared")
    agout1 = nc.dram_tensor("agout1", [N_CORES, 2, C, LLOC], F8, addr_space="Shared")

    rg = [list(range(N_CORES))]

    with tile.TileContext(nc) as tc:
        for _rep in range(reps):
            _emit_body(nc, tc, locals())
    nc.compile()
    return nc


def _emit_body(nc, tc, g):
    xT8, wdkv8, wdq8 = g["xT8"], g["wdkv8"], g["wdq8"]
    wk8, wq8, wv8, wo8 = g["wk8"], g["wq8"], g["wv8"], g["wo8"]
    CCd, SSd, CMd, ONESd, OUT = g["CCd"], g["SSd"], g["CMd"], g["ONESd"], g["OUT"]
    agin = (g["agin0"], g["agin1"])
    agout = (g["agout0"], g["agout1"])
    rg = g["rg"]
    Exp = mybir.ActivationFunctionType.Exp
    Copy = mybir.ActivationFunctionType.Copy
    MUL = mybir.AluOpType.mult
    SUB = mybir.AluOpType.subtract

    # PSUM pool, 8 banks:
    #   "s"   [128,2,BQ] x2 bufs = 4 banks (S^T groups; down-proj psums)
    #   "ctx" [128,BQ]   x2 bufs = 2 banks (PV accumulate; up-proj psums)
    #   "b1"  [128,BQ]   x1 buf  = 1 bank  (V psums; ones-denoms; out-proj)
    #   "o"   [128,BQ]   x1 buf  = 1 bank  (out-proj psums; swap-rope psums)
    with tc.tile_pool(name="sb_base", bufs=1) as sbB, tc.tile_pool(
        name="ps", bufs=1, space="PSUM"
    ) as psP:
        # one-time zero of every psum bank so stale regions read by
        # full-width exp groups are always finite
        if g.get("_first_rep", True):
            for _i in range(2):
                z = psP.tile([128, 2, BQ], F32, tag="s", bufs=2)
                nc.vector.memset(z[:], 0.0)
            for _i in range(2):
                z = psP.tile([128, BQ], F32, tag="ctx", bufs=2)
                nc.vector.memset(z[:], 0.0)
            z = psP.tile([128, BQ], F32, tag="b1", bufs=1)
            nc.vector.memset(z[:], 0.0)
            z = psP.tile([128, BQ], F32, tag="o", bufs=1)
            nc.vector.memset(z[:], 0.0)

        # persistent tiles
        kT8 = sbB.tile([128, 2, L], F8, tag="kT")     # [64p-per-head, rope/base, L]
        qT8 = sbB.tile([128, 2, L], F8, tag="qT")
        vN = sbB.tile([128, NKC, HPC * HD], BF16, tag="vN")   # holds SCTX*v
        ctx8h = sbB.tile([128, HPC, L], F8, tag="ctxh")
        ctx8l = sbB.tile([128, HPC, L], F8, tag="ctxl")
        wo_t = sbB.tile([128, 2, HPC, D], F8, tag="wo")       # [d, hi/lo, head, dout]
        cc_t = sbB.tile([128, L], BF16, tag="cc")
        ss_t = sbB.tile([128, L], BF16, tag="ss")
        cm_t = sbB.tile([128, 4, BQ], BF16, tag="cm")
        ones_t = sbB.tile([128, 1], BF16, tag="ones")
        # expS lives in the persistent pool so its one-time zeroing (needed
        # because diagonal-chunk exp skips the causally-dead columns) can run
        # on the idle Pool engine during phase 1
        if g.get("_first_rep", True):
            for _i in range(2):
                ez = sbB.tile([128, NKC, BQ], BF16, tag="expS", bufs=2)
                nc.gpsimd.memset(ez[:], 0.0)
        wk_t = sbB.tile([128, CKC, 3 * HPC * ROPE], F8, tag="wk")
        wq_t = sbB.tile([128, CKC, 3 * HPC * ROPE], F8, tag="wq")
        wv_t = sbB.tile([128, 2, CKC, HPC * HD], F8, tag="wv")

        # ---------------- Phase 1: down-projections + AllGathers ------------
        with tc.tile_pool(name="sb_dp", bufs=1) as sbD:
            xT_t = sbD.tile([128, 2, DKC, LLOC], F8, tag="xT")
            wdkv_t = sbD.tile([128, 2, DKC, C], F8, tag="wdkv")
            wdq_t = sbD.tile([128, 2, DKC, C], F8, tag="wdq")
            x4 = xT8.rearrange("hl (k p) l -> p hl k l", p=128)
            wdkv4 = wdkv8.rearrange("hl (k p) c -> p hl k c", p=128)
            wdq4 = wdq8.rearrange("hl (k p) c -> p hl k c", p=128)
            # hi parts first so the first matmul can start earliest
            nc.sync.dma_start(xT_t[:, 0:1, :, :], x4[:, 0:1, :, :])
            nc.sync.dma_start(wdkv_t[:, 0:1, :, :], wdkv4[:, 0:1, :, :])
            nc.sync.dma_start(xT_t[:, 1:2, :, :], x4[:, 1:2, :, :])
            nc.sync.dma_start(wdkv_t[:, 1:2, :, :], wdkv4[:, 1:2, :, :])
            nc.sync.dma_start(wdq_t[:], wdq4[:])
            # phase-2 weights next (they gate the first up-proj windows),
            # then rope constants, then phase-3-only constants last
            nc.sync.dma_start(wk_t[:], wk8.rearrange("(c p) m -> p c m", p=128))
            nc.sync.dma_start(wq_t[:], wq8.rearrange("(c p) m -> p c m", p=128))
            nc.sync.dma_start(wv_t[:], wv8.rearrange("hl (c p) m -> p hl c m", p=128))
            nc.sync.dma_start(cc_t[:], CCd[:])
            nc.sync.dma_start(ss_t[:], SSd[:])
            nc.sync.dma_start(ones_t[:], ONESd[:])
            nc.sync.dma_start(cm_t[:], CMd.rearrange("c p l -> p c l"))
            nc.sync.dma_start(wo_t[:], wo8.rearrange("hl (h p) d -> p hl h d", p=128))

            for gi_, w_t in enumerate((wdkv_t, wdq_t)):
                stage = sbD.tile([128, 2, CKC, LLOC], F8, tag="cstage")
                for t in range(CKC):
                    ps = psP.tile([128, LLOC], F32, tag="s", bufs=2)
                    mm = 0
                    # 3-term hi/lo: w_hi@x_hi, w_lo@x_hi, w_hi@x_lo
                    for whl, xhl in ((0, 0), (1, 0), (0, 1)):
                        for kp in range(DKC // 2):
                            nc.tensor.matmul(
                                ps[:],
                                w_t[:, whl, 2 * kp : 2 * kp + 2, t * 128 : (t + 1) * 128],
                                xT_t[:, xhl, 2 * kp : 2 * kp + 2, :],
                                start=(mm == 0),
                                stop=(mm == 3 * (DKC // 2) - 1),
                                perf_mode=DR,
                            )
                            mm += 1
                    nc.scalar.activation(stage[:, 0, t, :], ps[:], Copy, scale=F_CEV)
                    nc.vector.scalar_tensor_tensor(
                        out=stage[:, 1, t, :], in0=ps[:], scalar=F_CEV,
                        in1=stage[:, 0, t, :], op0=MUL, op1=SUB,
                    )
                nc.sync.dma_start(
                    agin[gi_].rearrange("hl (t p) l -> p hl t l", p=128), stage[:]
                )
                if not OPTS["no_ag"]:
                    nc.gpsimd.collective_compute(
                        "AllGather",
                        mybir.AluOpType.bypass,
                        replica_groups=rg,
                        ins=[agin[gi_][:]],
                        outs=[agout[gi_][:]],
                    )

        # ------- Phase 2+3: up-projections, attention, out-proj (merged) -----
        # Attention block 0 is interleaved right after Q-window 0 so the Act
        # engine's exp stream (the global bottleneck) starts as early as the
        # q AllGather allows; its out-projection is deferred to keep the
        # "o"/"b1" psum banks free during the remaining up-proj windows.
        sbAT = [None]
        with tc.tile_pool(name="sb_up", bufs=1) as sbU:

            def emit_proj(dst8, w_t, c8w, F, win):
                # psA: rope rows of both heads ([r_h0; r_h1]); psB: base rows;
                # psC: half-swapped rope rows (host-permuted weight columns)
                psA = psP.tile([128, BQ], F32, tag="ctx", bufs=2)
                psB = psP.tile([128, BQ], F32, tag="ctx", bufs=2)
                psC = psP.tile([128, BQ], F32, tag="o", bufs=1)
                for half, ps in ((0, psA), (1, psB), (2, psC)):
                    for cp in range(2):
                        nc.tensor.matmul(
                            ps[:],
                            w_t[:, 2 * cp : 2 * cp + 2, half * 128 : (half + 1) * 128],
                            c8w[:, 0, 2 * cp : 2 * cp + 2, :],
                            start=(cp == 0),
                            stop=(cp == 1),
                            perf_mode=DR,
                        )
                # cc/ss tables carry the psum->dst scale F (folded on host)
                swm = sbU.tile([128, BQ], BF16, tag="swm", bufs=3)
                nc.vector.tensor_mul(swm[:], psC[:], ss_t[:, win])
                t_r = sbU.tile([128, BQ], BF16, tag="tr", bufs=3)
                nc.vector.tensor_mul(t_r[:], psA[:], cc_t[:, win])
                nc.vector.tensor_add(dst8[:, 0, win], t_r[:], swm[:])
                nc.scalar.activation(dst8[:, 1, win], psB[:], Copy, scale=F)

            def emit_passA(w):
                win = slice(w * BQ, (w + 1) * BQ)
                ckw = sbU.tile([128, 2, CKC, BQ], F8, tag="ckw", bufs=5)
                nc.sync.dma_start(
                    ckw[:], agout[0][w].rearrange("hl (t p) l -> p hl t l", p=128)
                )
                emit_proj(kT8, wk_t, ckw, F_K, win)
                # V: natural layout [Lk, d] chunks; 3-term hi/lo
                for j in range(4):
                    lc = w * 4 + j
                    ps = psP.tile([128, HPC * HD], F32, tag="b1", bufs=1)
                    mm = 0
                    for chl, whl in ((0, 0), (1, 0), (0, 1)):
                        for cp in range(2):
                            nc.tensor.matmul(
                                ps[:],
                                ckw[:, chl, 2 * cp : 2 * cp + 2, j * 128 : (j + 1) * 128],
                                wv_t[:, whl, 2 * cp : 2 * cp + 2, :],
                                start=(mm == 0),
                                stop=(mm == 5),
                                perf_mode=DR,
                            )
                            mm += 1
                    nc.scalar.activation(vN[:, lc, :], ps[:], Copy, scale=F_V)

            def emit_passB(w):
                win = slice(w * BQ, (w + 1) * BQ)
                cqw = sbU.tile([128, 2, CKC, BQ], F8, tag="cqw", bufs=5)
                nc.sync.dma_start(
                    cqw[:], agout[1][w].rearrange("hl (t p) l -> p hl t l", p=128)
                )
                emit_proj(qT8, wq_t, cqw, F_Q, win)

            def attn_block(b):
                nch = 4 * (b + 1)
                qwin = slice(b * BQ, (b + 1) * BQ)
                use_ones = (b in ONES_BLOCKS) and not OPTS["no_denom"]
                for h in range(HPC):
                    hsl = slice(64 * h, 64 * (h + 1))
                    expS = sbB.tile([128, NKC, BQ], BF16, tag="expS", bufs=2)
                    dsum = sbAT[0].tile([128, BQ], F32, tag="dsum", bufs=3)
                    ctx_ps = psP.tile([128, BQ], F32, tag="ctx", bufs=2)
                    if use_ones:
                        ones_ps = psP.tile([128, BQ], F32, tag="b1", bufs=1)

                    def emit_pv(ck0, gsz):
                        for j in range(gsz):
                            ck = ck0 + j
                            off = max(0, (ck - (nch - 4)) * 128)
                            nc.tensor.matmul(
                                ctx_ps[:, off:BQ],
                                vN[:, ck, h * HD : (h + 1) * HD],
                                expS[:, ck, off:BQ],
                                start=(ck == 0),
                                stop=(ck == nch - 1),
                            )

                    # software-pipelined by one group: PV(g) emitted after
                    # scores(g+1) so the in-order PE queue never stalls
                    prev_group = None
                    ck0 = 0
                    while ck0 < nch:
                        gsz = min(2, nch - ck0)
                        s_ps = psP.tile([128, 2, BQ], F32, tag="s", bufs=2)
                        for j in range(gsz):
                            ck = ck0 + j
                            off = max(0, (ck - (nch - 4)) * 128)
                            nc.tensor.matmul(
                                s_ps[:, j, off:BQ],
                                kT8[hsl, :, ck * 128 : (ck + 1) * 128],
                                qT8[hsl, :, b * BQ + off : (b + 1) * BQ],
                                start=True,
                                stop=True,
                                perf_mode=DR,
                            )
                        if prev_group is not None:
                            emit_pv(*prev_group)
                        # exp only the causally-live columns of the group
                        # (off of the group's FIRST chunk; the mask zeroes the
                        # [off0:off1] sliver of the second chunk)
                        off0 = max(0, (ck0 - (nch - 4)) * 128)
                        nc.scalar.activation(
                            expS[:, ck0 : ck0 + gsz, off0:BQ],
                            s_ps[:, 0:gsz, off0:BQ],
                            Exp,
                            scale=F_EXP,
                        )
                        for j in range(gsz):
                            ck = ck0 + j
                            if ck >= nch - 4:  # diagonal chunk: causal mask
                                off = max(0, (ck - (nch - 4)) * 128)
                                nc.vector.tensor_mul(
                                    expS[:, ck, 0 : off + 128],
                                    expS[:, ck, 0 : off + 128],
                                    cm_t[:, ck - (nch - 4), 0 : off + 128],
                                )
                        if use_ones:
                            for j in range(gsz):
                                ck = ck0 + j
                                off = max(0, (ck - (nch - 4)) * 128)
                                nc.tensor.matmul(
                                    ones_ps[0:1, off:BQ],
                                    ones_t[:, 0:1],
                                    expS[:, ck, off:BQ],
                                    start=(ck == 0),
                                    stop=(ck == nch - 1),
                                )
                        elif not OPTS["no_denom"] and ck0 % 4 == 2:
                            # pair/quad bf16 tree on DVE
                            epair2 = sbAT[0].tile([128, 2, BQ], BF16, tag="epair2", bufs=2)
                            nc.vector.tensor_add(
                                epair2[:],
                                expS[:, ck0 - 2 : ck0, :],
                                expS[:, ck0 : ck0 + 2, :],
                            )
                            if ck0 == 2:
                                nc.vector.tensor_add(
                                    dsum[:], epair2[:, 0, :], epair2[:, 1, :]
                                )
                            else:
                                equad = sbAT[0].tile([128, BQ], BF16, tag="equad", bufs=2)
                                nc.vector.tensor_add(
                                    equad[:], epair2[:, 0, :], epair2[:, 1, :]
                                )
                                nc.vector.tensor_add(dsum[:], dsum[:], equad[:])
                        prev_group = (ck0, gsz)
                        ck0 += gsz
                    emit_pv(*prev_group)
                    if OPTS["no_denom"]:
                        nc.vector.tensor_copy(ctx8h[:, h, qwin], ctx_ps[:])
                        continue
                    bc_t = sbAT[0].tile([128, BQ], F32, tag="bc_t", bufs=3)
                    dsb = sbAT[0].tile([128, BQ], F32, tag="dsb", bufs=3)
                    if use_ones:
                        nc.vector.reciprocal_approx_fast(
                            out=dsb[0:1, :], in_=ones_ps[0:1, :]
                        )
                        nc.gpsimd.partition_broadcast(bc_t[:], dsb[0:1, :])
                    else:
                        nc.gpsimd.partition_all_reduce(
                            dsb[:], dsum[:], channels=128, reduce_op=_ReduceOp.add
                        )
                        nc.vector.reciprocal_approx_fast(out=bc_t[:], in_=dsb[:])
                    t_bf = sbAT[0].tile([128, BQ], BF16, tag="tbf", bufs=3)
                    nc.vector.tensor_mul(t_bf[:], ctx_ps[:], bc_t[:])
                    # hi/lo fp8 split of ctx on the Pool engine
                    nc.gpsimd.tensor_copy(ctx8h[:, h, qwin], t_bf[:])
                    nc.gpsimd.tensor_sub(
                        ctx8l[:, h, qwin], t_bf[:], ctx8h[:, h, qwin]
                    )

            def outproj_block(b):
                # fused out-projection for a q-window (4 row-chunks of 128)
                for j in range(4):
                    lc = b * 4 + j
                    lsl = slice(lc * 128, (lc + 1) * 128)
                    ostage = sbAT[0].tile([128, D], BF16, tag="ostage", bufs=5)
                    for do in range(4):
                        ps = psP.tile(
                            [128, 512], F32, tag=("o" if do % 2 == 0 else "b1"),
                            bufs=1,
                        )
                        osl = slice(do * 512, (do + 1) * 512)
                        for mi, (cx, whl) in enumerate(
                            ((ctx8h, 0), (ctx8l, 0), (ctx8h, 1))
                        ):
                            nc.tensor.matmul(
                                ps[:],
                                cx[:, :, lsl],
                                wo_t[:, whl, :, osl],
                                start=(mi == 0),
                                stop=(mi == 2),
                                perf_mode=DR,
                            )
                        if do % 2 == 0:
                            nc.vector.tensor_scalar_mul(ostage[:, osl], ps[:], F_O)
                        else:
                            nc.scalar.activation(ostage[:, osl], ps[:], Copy, scale=F_O)
                    nc.sync.dma_start(OUT[lsl, :], ostage[:])

            if not OPTS["no_upproj"]:
                for w in range(NB):
                    emit_passA(w)
                for w in range(NB):
                    emit_passB(w)
        if not OPTS["no_attn"]:
            with tc.tile_pool(name="sb_at", bufs=1) as _sbat:
                sbAT[0] = _sbat
                for b in (0, 7, 6, 5, 4, 3, 2, 1):
                    attn_block(b)
                    if not OPTS["no_outproj"]:
                        outproj_block(b)


def _hilo8(a, s):
    h = (a * s).astype(E4)
    lo = (a * s - h.astype(np.float32)).astype(E4)
    return h, lo


def _host_inputs(x, W_dkv, W_dq, W_uk, W_uv, W_uq, W_qr, W_kr, W_out):
    """Build per-core input maps (numpy)."""
    scale = 1.0 / math.sqrt(HD)

    xt = np.ascontiguousarray(x.reshape(L, D).T)  # [D, L] f32

    # rope tables, transposed: ang[l, i] = l * inv_freq[i]
    inv_freq = 1.0 / (ROPE_BASE ** (np.arange(HALF, dtype=np.float64) * 2.0 / ROPE))
    ang = np.arange(L, dtype=np.float64)[:, None] * inv_freq[None, :]  # [L, 32]
    cosT = np.cos(ang).T.astype(np.float32)  # [32, L]
    sinT = np.sin(ang).T.astype(np.float32)
    cc64 = np.concatenate([cosT, cosT], axis=0)        # [64, L]
    ss64 = np.concatenate([-sinT, sinT], axis=0)       # [64, L]
    # F_K (== F_Q) psum->k8 scale folds into the rope tables
    CCm = np.concatenate([cc64, cc64], axis=0) * F_K   # [128, L] (both head slots)
    SSm = np.concatenate([ss64, ss64], axis=0) * F_K

    CM = np.zeros((4, 128, BQ), dtype=np.float32)
    for j in range(4):
        for lk in range(128):
            CM[j, lk, j * 128 + lk :] = 1.0

    # rope half-swap as a column permutation of the rope projection weights
    swap_idx = np.concatenate(
        [np.array([g * 64 + (m + HALF) % 64 for m in range(64)]) for g in range(2)]
    )

    bf = lambda a: np.ascontiguousarray(a).astype(ml_dtypes.bfloat16)
    ONES = np.ones((128, 1), dtype=np.float32)

    wdkv_h, wdkv_l = _hilo8(W_dkv, SWD)
    wdq_h, wdq_l = _hilo8(W_dq, SWD)
    wdkv8 = np.stack([wdkv_h, wdkv_l])
    wdq8 = np.stack([wdq_h, wdq_l])

    in_maps = []
    for r in range(N_CORES):
        heads = [2 * r, 2 * r + 1]
        # column layout: [rope_h0 | rope_h1 | base_h0 | base_h1] (64 each)
        kr = [W_kr[:, hh * ROPE : (hh + 1) * ROPE] for hh in heads]
        kb = [W_uk[:, hh * SPLIT : (hh + 1) * SPLIT] for hh in heads]
        qr = [W_qr[:, hh * ROPE : (hh + 1) * ROPE] * scale for hh in heads]
        qb = [W_uq[:, hh * SPLIT : (hh + 1) * SPLIT] * scale for hh in heads]
        kr_blk = np.concatenate(kr, axis=1)
        qr_blk = np.concatenate(qr, axis=1)
        # [rope | base | rope-halfswapped]
        wk_cols = np.concatenate([kr_blk] + kb + [kr_blk[:, swap_idx]], axis=1)
        wq_cols = np.concatenate([qr_blk] + qb + [qr_blk[:, swap_idx]], axis=1)
        wv_cols = np.concatenate(
            [W_uv[:, hh * HD : (hh + 1) * HD] for hh in heads], axis=1
        )
        wo_rows = np.concatenate(
            [W_out[hh * HD : (hh + 1) * HD, :] for hh in heads], axis=0
        )
        xh, xl = _hilo8(xt[:, r * LLOC : (r + 1) * LLOC], SX)
        wvh, wvl = _hilo8(wv_cols, SWV)
        woh, wol = _hilo8(wo_rows, SWO)
        in_maps.append(
            {
                "xT8": np.stack([xh, xl]),
                "wdkv8": wdkv8,
                "wdq8": wdq8,
                "wk8": (wk_cols * SWK).astype(E4),
                "wq8": (wq_cols * SWQ).astype(E4),
                "wv8": np.stack([wvh, wvl]),
                "wo8": np.stack([woh, wol]),
                "CC": bf(CCm),
                "SS": bf(SSm),
                "CM": bf(CM),
                "ONES": bf(ONES),
            }
        )
    return in_maps


def _get_program(reps=1):
    if reps not in _CACHE:
        _CACHE[reps] = _build_program(reps)
    return _CACHE[reps]


def make_runner(in_maps, reps=1, empty=False):
    """Persistent compiled runner for timing: returns run_chain(M) that executes
    the program M times back-to-back on device (chained via the output buffer so
    executions serialize), returning wall seconds for the chain."""
    import time as _time
    import jax
    from jax.sharding import Mesh, PartitionSpec, NamedSharding
    from jax.experimental.shard_map import shard_map
    import concourse.bass2jax as bass2jax

    nc = _EMPTY_CACHE.setdefault(0, _build_empty()) if empty else _get_program(reps)
    bass2jax.install_neuronx_cc_hook()
    partition_name = nc.partition_id_tensor.name if nc.partition_id_tensor else None
    in_names, out_names, out_avals, zero_outs = [], [], [], []
    for alloc in nc.m.functions[0].allocations:
        if not isinstance(alloc, mybir.MemoryLocationSet):
            continue
        name = alloc.memorylocations[0].name
        if alloc.kind == "ExternalInput":
            if name != partition_name:
                in_names.append(name)
        elif alloc.kind == "ExternalOutput":
            out_names.append(name)
            shape = tuple(alloc.tensor_shape)
            dtype = mybir.dt.np(alloc.dtype)
            out_avals.append(jax.core.ShapedArray(shape, dtype))
            zero_outs.append(np.zeros(shape, dtype))
    n_params = len(in_names)
    in_names_all = in_names + out_names
    if partition_name is not None:
        in_names_all = in_names_all + [partition_name]

    def _body(*args):
        operands = list(args)
        if partition_name is not None:
            operands.append(bass2jax.partition_id_tensor())
        outs = bass2jax._bass_exec_p.bind(
            *operands,
            out_avals=tuple(out_avals),
            in_names=tuple(in_names_all),
            out_names=tuple(out_names),
            lowering_input_output_aliases=(),
            sim_require_finite=True,
            sim_require_nnan=True,
            nc=nc,
        )
        return tuple(outs)

    devices = jax.devices()[:N_CORES]
    mesh = Mesh(np.asarray(devices), ("core",))
    n_outs = len(out_names)
    in_specs = (PartitionSpec("core"),) * (n_params + n_outs)
    out_specs = (PartitionSpec("core"),) * n_outs
    sharded = jax.jit(
        shard_map(_body, mesh=mesh, in_specs=in_specs, out_specs=out_specs, check_rep=False),
        keep_unused=True,
    )
    sh = NamedSharding(mesh, PartitionSpec("core"))
    concat_in = [
        np.concatenate([np.asarray(in_maps[c][nm]) for c in range(N_CORES)], axis=0)
        for nm in in_names
    ]
    concat_zeros = [
        np.zeros((N_CORES * z.shape[0], *z.shape[1:]), z.dtype) for z in zero_outs
    ]
    dev_in = [jax.device_put(a, sh) for a in concat_in]
    dev_zero = [jax.device_put(a, sh) for a in concat_zeros]
    outs = sharded(*dev_in, *dev_zero)
    jax.block_until_ready(outs)  # compile + warm

    def run_chain(M):
        z = list(dev_zero)
        t0 = _time.perf_counter()
        outs = None
        for _ in range(M):
            outs = sharded(*dev_in, *z)
            z = list(outs)
        jax.block_until_ready(outs)
        return _time.perf_counter() - t0

    return run_chain


_EMPTY_CACHE = {}


def _build_empty():
    """Minimal program with the same I/O signature class: one tiny DMA."""
    nc = bacc.Bacc("TRN2", target_bir_lowering=False, debug=False, num_devices=N_CORES)
    xT = nc.dram_tensor("xT", [128, 128], F32, kind="ExternalInput")
    OUT = nc.dram_tensor("OUT", [128, 128], F32, kind="ExternalOutput")
    with tile.TileContext(nc) as tc:
        with tc.tile_pool(name="sb", bufs=1) as sb:
            t = sb.tile([128, 128], F32, tag="t")
            nc.sync.dma_start(t[:], xT[:])
            nc.sync.dma_start(OUT[:], t[:])
    nc.compile()
    return nc


def make_empty_runner():
    in_maps = [{"xT": np.zeros((128, 128), np.float32)} for _ in range(N_CORES)]
    return make_runner(in_maps, empty=True)


def kernel(x, W_dkv, W_dq, W_uk, W_uv, W_uq, W_qr, W_kr, W_out, b_out, reps=1):
    x = np.asarray(x, dtype=np.float32)
    in_maps = _host_inputs(
        x,
        np.asarray(W_dkv, np.float32), np.asarray(W_dq, np.float32),
        np.asarray(W_uk, np.float32), np.asarray(W_uv, np.float32),
        np.asarray(W_uq, np.float32), np.asarray(W_qr, np.float32),
        np.asarray(W_kr, np.float32), np.asarray(W_out, np.float32),
    )
    nc = _get_program(reps)
    res = run_bass_kernel_spmd(nc, in_maps, core_ids=list(range(N_CORES)), trace=False)
    out = np.zeros((L, D), dtype=np.float32)
    for r in range(N_CORES):
        out += res.results[r]["OUT"].astype(np.float32)
    out += np.asarray(b_out, np.float32)[None, :]
    return out.reshape(1, L, D)


# revision 41
# speedup vs baseline: 1.2022x; 1.2022x over previous
"""Multi-Head Latent Attention (MLA) forward on 8 Trainium2 NeuronCores.

Contract: kernel(**inputs) takes the FULL unsharded inputs (numpy) and
returns the FULL [1, 4096, 2048] float32 output.

Sharding (hardcoded):
  - Tensor-parallel over heads: 2 heads per core (up-proj weights
    column-sharded, W_out row-sharded; partial outputs summed on host).
  - Down-projections (x @ W_dkv, x @ W_dq) are sharded over the sequence
    (512 rows per core) and AllGathered as fp8 hi+lo pairs.

fp8 strategy (e4m3, all tensors pre-scaled by power-of-2 so rms ~10):
  - down-proj: x and W_d both hi+lo fp8, 3-term DoubleRow (drop lo*lo)
    -> bf16-grade c at 0.75x bf16 PE cost.
  - c is evacuated as hi+lo fp8 and AllGathered packed ([2, C, LLOC]).
  - K/Q up-proj: single-fp8 (c_hi @ w_hi) DoubleRow -> 0.25x PE cost
    (softmax forgives the ~8% noise; validated 6.8e-3 rel err).
  - V up-proj: 3-term hi/lo DoubleRow (V accuracy matters) -> 0.75x.
  - scores: q/k stored fp8 [64p, 2(rope/base), L] per head (head h at
    partitions 64h..64h+64); DoubleRow contracts rope+base -> 0.5x.
  - PV + softmax: bf16 (fp8 there fails the accuracy budget).
  - out-proj: ctx split hi/lo on device (Pool engine), W_out hi/lo on
    host, 3-term DoubleRow -> 0.75x.
  - denominators: exact; per-block either DVE pair/quad tree (small
    blocks) or a bf16 ones-matmul column-sum on PE (large blocks,
    ONES_BLOCKS) -- PE reductions are ~3x cheaper than DVE here.

Device layout notes:
  - All activations transposed ([dim, L]); no on-device transposes.
  - RoPE: rope rows of both heads packed at psum A ([r_h0;r_h1]), base at
    psum B; the half-swap is a THIRD projection psC whose weight columns
    are permuted on the host (costs 2 fp8 matmuls/window, removes the
    Act-copy + perm-matmul chain); the psum->k8/q8 evac scale F_K folds
    into the host cos/sin tables (SQ chosen so F_Q == F_K).
  - Attention in S^T = [Lk, Lq] orientation, causally-restricted column
    tails on diagonal chunks; exp descale (1/(SQ*SK)) folds into the Act
    activation scale; SCTX folds into the V evac so normalize needs no
    extra scaling.
  - Scores are small (|s| < ~2), so exp() runs without max-subtraction.
"""

import sys

for _p in ("/opt/trn_rl_repo", "/opt/pypackages"):
    if _p not in sys.path:
        sys.path.insert(0, _p)

import math
import numpy as np
import ml_dtypes

import concourse.bacc as bacc
import concourse.mybir as mybir
import concourse.tile as tile
from concourse.bass_isa import ReduceOp as _ReduceOp
from concourse.bass_utils import run_bass_kernel_spmd

# Problem constants
L = 4096
D = 2048
C = 512
H = 16
HD = 128          # head dim
ROPE = 64
HALF = ROPE // 2  # 32
SPLIT = HD - ROPE # 64
N_CORES = 8
HPC = H // N_CORES   # heads per core = 2
LLOC = L // N_CORES  # 512 (down-proj shard)
BQ = 512             # Lq block
NB = L // BQ         # 8
NKC = L // 128       # 32 Lk chunks
DKC = D // 128       # 16
CKC = C // 128       # 4
ROPE_BASE = 10000.0

BF16 = mybir.dt.bfloat16
F32 = mybir.dt.float32
F8 = mybir.dt.float8e4
E4 = ml_dtypes.float8_e4m3
DR = mybir.MatmulPerfMode.DoubleRow

# fp8 power-of-2 scales (chosen so each quantized tensor has rms ~5-15,
# safely inside e4m3's [2^-6, 240] normal range)
SX, SWD, SC = 16.0, 256.0, 16.0
SWK, SWQ, SK, SQ = 256.0, 4096.0, 32.0, 512.0
SWV, SCTX, SWO = 256.0, 16.0, 256.0
F_K = SK / (SC * SWK)       # K up psum -> k8           (1/128)
F_Q = SQ / (SC * SWQ)       # Q up psum -> q8           (1/128, == F_K)
assert F_K == F_Q           # shared folded cc/ss tables require this
F_CEV = SC / (SX * SWD)     # down-proj psum -> c8      (1/256)
F_V = SCTX / (SC * SWV)     # V up psum -> vN (bf16)    (1/256)
F_O = 1.0 / (SCTX * SWO)    # out psum -> OUT (bf16)    (1/4096)
F_EXP = 1.0 / (SQ * SK)     # scores psum descale in exp (1/16384)

# blocks whose softmax denominator is computed by PE ones-matmul instead
# of the DVE pair/quad tree (engine balancing)
ONES_BLOCKS = (6, 7)

_CACHE = {}

# Ablation flags for subtractive profiling (timing only; output garbage when set)
OPTS = {
    "no_attn": False,
    "no_outproj": False,
    "no_ag": False,
    "no_denom": False,
    "no_upproj": False,
}


def _build_program(reps=1):
    nc = bacc.Bacc("TRN2", target_bir_lowering=False, debug=False, num_devices=N_CORES)

    xT8 = nc.dram_tensor("xT8", [2, D, LLOC], F8, kind="ExternalInput")
    wdkv8 = nc.dram_tensor("wdkv8", [2, D, C], F8, kind="ExternalInput")
    wdq8 = nc.dram_tensor("wdq8", [2, D, C], F8, kind="ExternalInput")
    # cols: [rope(128) | base(128) | rope-halfswapped(128)]
    wk8 = nc.dram_tensor("wk8", [C, 3 * HPC * ROPE], F8, kind="ExternalInput")
    wq8 = nc.dram_tensor("wq8", [C, 3 * HPC * ROPE], F8, kind="ExternalInput")
    wv8 = nc.dram_tensor("wv8", [2, C, HPC * HD], F8, kind="ExternalInput")
    wo8 = nc.dram_tensor("wo8", [2, HPC * HD, D], F8, kind="ExternalInput")
    CCd = nc.dram_tensor("CC", [128, L], BF16, kind="ExternalInput")
    SSd = nc.dram_tensor("SS", [128, L], BF16, kind="ExternalInput")
    CMd = nc.dram_tensor("CM", [4, 128, BQ], BF16, kind="ExternalInput")
    ONESd = nc.dram_tensor("ONES", [128, 1], BF16, kind="ExternalInput")
    OUT = nc.dram_tensor("OUT", [L, D], BF16, kind="ExternalOutput")

    agin0 = nc.dram_tensor("agin0", [2, C, LLOC], F8)
    agin1 = nc.dram_tensor("agin1", [2, C, LLOC], F8)
    agout0 = nc.dram_tensor("agout0", [N_CORES, 2, C, LLOC], F8, addr_space="Shared")
    agout1 = nc.dram_tensor("agout1", [N_CORES, 2, C, LLOC], F8, addr_space="Shared")

    rg = [list(range(N_CORES))]

    with tile.TileContext(nc) as tc:
        for _rep in range(reps):
            _emit_body(nc, tc, locals())
    nc.compile()
    return nc


def _emit_body(nc, tc, g):
    xT8, wdkv8, wdq8 = g["xT8"], g["wdkv8"], g["wdq8"]
    wk8, wq8, wv8, wo8 = g["wk8"], g["wq8"], g["wv8"], g["wo8"]
    CCd, SSd, CMd, ONESd, OUT = g["CCd"], g["SSd"], g["CMd"], g["ONESd"], g["OUT"]
    agin = (g["agin0"], g["agin1"])
    agout = (g["agout0"], g["agout1"])
    rg = g["rg"]
    Exp = mybir.ActivationFunctionType.Exp
    Copy = mybir.ActivationFunctionType.Copy
    MUL = mybir.AluOpType.mult
    SUB = mybir.AluOpType.subtract

    # PSUM pool, 8 banks:
    #   "s"   [128,2,BQ] x2 bufs = 4 banks (S^T groups; down-proj psums)
    #   "ctx" [128,BQ]   x2 bufs = 2 banks (PV accumulate; up-proj psums)
    #   "b1"  [128,BQ]   x1 buf  = 1 bank  (V psums; ones-denoms; out-proj)
    #   "o"   [128,BQ]   x1 buf  = 1 bank  (out-proj psums; swap-rope psums)
    with tc.tile_pool(name="sb_base", bufs=1) as sbB, tc.tile_pool(
        name="ps", bufs=1, space="PSUM"
    ) as psP:
        # one-time zero of every psum bank so stale regions read by
        # full-width exp groups are always finite
        if g.get("_first_rep", True):
            for _i in range(2):
                z = psP.tile([128, 2, BQ], F32, tag="s", bufs=2)
                nc.vector.memset(z[:], 0.0)
            for _i in range(2):
                z = psP.tile([128, BQ], F32, tag="ctx", bufs=2)
                nc.vector.memset(z[:], 0.0)
            z = psP.tile([128, BQ], F32, tag="b1", bufs=1)
            nc.vector.memset(z[:], 0.0)
            z = psP.tile([128, BQ], F32, tag="o", bufs=1)
            nc.vector.memset(z[:], 0.0)

        # persistent tiles
        kT8 = sbB.tile([128, 2, L], F8, tag="kT")     # [64p-per-head, rope/base, L]
        qT8 = sbB.tile([128, 2, L], F8, tag="qT")
        vN = sbB.tile([128, NKC, HPC * HD], BF16, tag="vN")   # holds SCTX*v
        ctx8h = sbB.tile([128, HPC, L], F8, tag="ctxh")
        ctx8l = sbB.tile([128, HPC, L], F8, tag="ctxl")
        wo_t = sbB.tile([128, 2, HPC, D], F8, tag="wo")       # [d, hi/lo, head, dout]
        cc_t = sbB.tile([128, L], BF16, tag="cc")
        ss_t = sbB.tile([128, L], BF16, tag="ss")
        cm_t = sbB.tile([128, 4, BQ], BF16, tag="cm")
        ones_t = sbB.tile([128, 1], BF16, tag="ones")
        # expS lives in the persistent pool so its one-time zeroing (needed
        # because diagonal-chunk exp skips the causally-dead columns) can run
        # on the idle Pool engine during phase 1
        if g.get("_first_rep", True):
            for _i in range(2):
                ez = sbB.tile([128, NKC, BQ], BF16, tag="expS", bufs=2)
                nc.gpsimd.memset(ez[:], 0.0)
        wk_t = sbB.tile([128, CKC, 3 * HPC * ROPE], F8, tag="wk")
        wq_t = sbB.tile([128, CKC, 3 * HPC * ROPE], F8, tag="wq")
        wv_t = sbB.tile([128, 2, CKC, HPC * HD], F8, tag="wv")

        # ---------------- Phase 1: down-projections + AllGathers ------------
        with tc.tile_pool(name="sb_dp", bufs=1) as sbD:
            xT_t = sbD.tile([128, 2, DKC, LLOC], F8, tag="xT")
            wdkv_t = sbD.tile([128, 2, DKC, C], F8, tag="wdkv")
            wdq_t = sbD.tile([128, 2, DKC, C], F8, tag="wdq")
            x4 = xT8.rearrange("hl (k p) l -> p hl k l", p=128)
            wdkv4 = wdkv8.rearrange("hl (k p) c -> p hl k c", p=128)
            wdq4 = wdq8.rearrange("hl (k p) c -> p hl k c", p=128)
            # hi parts first so the first matmul can start earliest
            nc.sync.dma_start(xT_t[:, 0:1, :, :], x4[:, 0:1, :, :])
            nc.sync.dma_start(wdkv_t[:, 0:1, :, :], wdkv4[:, 0:1, :, :])
            nc.sync.dma_start(xT_t[:, 1:2, :, :], x4[:, 1:2, :, :])
            nc.sync.dma_start(wdkv_t[:, 1:2, :, :], wdkv4[:, 1:2, :, :])
            nc.sync.dma_start(wdq_t[:], wdq4[:])
            # phase-2 weights next (they gate the first up-proj windows),
            # then rope constants, then phase-3-only constants last
            nc.sync.dma_start(wk_t[:], wk8.rearrange("(c p) m -> p c m", p=128))
            nc.sync.dma_start(wq_t[:], wq8.rearrange("(c p) m -> p c m", p=128))
            nc.sync.dma_start(wv_t[:], wv8.rearrange("hl (c p) m -> p hl c m", p=128))
            nc.sync.dma_start(cc_t[:], CCd[:])
            nc.sync.dma_start(ss_t[:], SSd[:])
            nc.sync.dma_start(ones_t[:], ONESd[:])
            nc.sync.dma_start(cm_t[:], CMd.rearrange("c p l -> p c l"))
            nc.sync.dma_start(wo_t[:], wo8.rearrange("hl (h p) d -> p hl h d", p=128))

            for gi_, w_t in enumerate((wdkv_t, wdq_t)):
                stage = sbD.tile([128, 2, CKC, LLOC], F8, tag="cstage")
                for t in range(CKC):
                    ps = psP.tile([128, LLOC], F32, tag="s", bufs=2)
                    mm = 0
                    # 3-term hi/lo: w_hi@x_hi, w_lo@x_hi, w_hi@x_lo
                    for whl, xhl in ((0, 0), (1, 0), (0, 1)):
                        for kp in range(DKC // 2):
                            nc.tensor.matmul(
                                ps[:],
                                w_t[:, whl, 2 * kp : 2 * kp + 2, t * 128 : (t + 1) * 128],
                                xT_t[:, xhl, 2 * kp : 2 * kp + 2, :],
                                start=(mm == 0),
                                stop=(mm == 3 * (DKC // 2) - 1),
                                perf_mode=DR,
                            )
                            mm += 1
                    nc.scalar.activation(stage[:, 0, t, :], ps[:], Copy, scale=F_CEV)
                    nc.vector.scalar_tensor_tensor(
                        out=stage[:, 1, t, :], in0=ps[:], scalar=F_CEV,
                        in1=stage[:, 0, t, :], op0=MUL, op1=SUB,
                    )
                nc.sync.dma_start(
                    agin[gi_].rearrange("hl (t p) l -> p hl t l", p=128), stage[:]
                )
                if not OPTS["no_ag"]:
                    nc.gpsimd.collective_compute(
                        "AllGather",
                        mybir.AluOpType.bypass,
                        replica_groups=rg,
                        ins=[agin[gi_][:]],
                        outs=[agout[gi_][:]],
                    )

        # ------- Phase 2+3: up-projections, attention, out-proj (merged) -----
        # Attention block 0 is interleaved right after Q-window 0 so the Act
        # engine's exp stream (the global bottleneck) starts as early as the
        # q AllGather allows; its out-projection is deferred to keep the
        # "o"/"b1" psum banks free during the remaining up-proj windows.
        sbAT = [None]
        with tc.tile_pool(name="sb_up", bufs=1) as sbU:

            def emit_proj(dst8, w_t, c8w, F, win):
                # psA: rope rows of both heads ([r_h0; r_h1]); psB: base rows;
                # psC: half-swapped rope rows (host-permuted weight columns)
                psA = psP.tile([128, BQ], F32, tag="ctx", bufs=2)
                psB = psP.tile([128, BQ], F32, tag="ctx", bufs=2)
                psC = psP.tile([128, BQ], F32, tag="o", bufs=1)
                for half, ps in ((0, psA), (1, psB), (2, psC)):
                    for cp in range(2):
                        nc.tensor.matmul(
                            ps[:],
                            w_t[:, 2 * cp : 2 * cp + 2, half * 128 : (half + 1) * 128],
                            c8w[:, 0, 2 * cp : 2 * cp + 2, :],
                            start=(cp == 0),
                            stop=(cp == 1),
                            perf_mode=DR,
                        )
                # cc/ss tables carry the psum->dst scale F (folded on host)
                swm = sbU.tile([128, BQ], BF16, tag="swm", bufs=3)
                nc.vector.tensor_mul(swm[:], psC[:], ss_t[:, win])
                t_r = sbU.tile([128, BQ], BF16, tag="tr", bufs=3)
                nc.vector.tensor_mul(t_r[:], psA[:], cc_t[:, win])
                nc.vector.tensor_add(dst8[:, 0, win], t_r[:], swm[:])
                nc.scalar.activation(dst8[:, 1, win], psB[:], Copy, scale=F)

            def emit_passA(w):
                win = slice(w * BQ, (w + 1) * BQ)
                ckw = sbU.tile([128, 2, CKC, BQ], F8, tag="ckw", bufs=5)
                nc.sync.dma_start(
                    ckw[:], agout[0][w].rearrange("hl (t p) l -> p hl t l", p=128)
                )
                emit_proj(kT8, wk_t, ckw, F_K, win)
                # V: natural layout [Lk, d] chunks; 3-term hi/lo
                for j in range(4):
                    lc = w * 4 + j
                    ps = psP.tile([128, HPC * HD], F32, tag="b1", bufs=1)
                    mm = 0
                    for chl, whl in ((0, 0), (1, 0), (0, 1)):
                        for cp in range(2):
                            nc.tensor.matmul(
                                ps[:],
                                ckw[:, chl, 2 * cp : 2 * cp + 2, j * 128 : (j + 1) * 128],
                                wv_t[:, whl, 2 * cp : 2 * cp + 2, :],
                                start=(mm == 0),
                                stop=(mm == 5),
                                perf_mode=DR,
                            )
                            mm += 1
                    nc.scalar.activation(vN[:, lc, :], ps[:], Copy, scale=F_V)

            def emit_passB(w):
                win = slice(w * BQ, (w + 1) * BQ)
                cqw = sbU.tile([128, 2, CKC, BQ], F8, tag="cqw", bufs=5)
                nc.sync.dma_start(
                    cqw[:], agout[1][w].rearrange("hl (t p) l -> p hl t l", p=128)
                )
                emit_proj(qT8, wq_t, cqw, F_Q, win)

            def attn_block(b):
                nch = 4 * (b + 1)
                qwin = slice(b * BQ, (b + 1) * BQ)
                use_ones = (b in ONES_BLOCKS) and not OPTS["no_denom"]
                for h in range(HPC):
                    hsl = slice(64 * h, 64 * (h + 1))
                    expS = sbB.tile([128, NKC, BQ], BF16, tag="expS", bufs=2)
                    dsum = sbAT[0].tile([128, BQ], F32, tag="dsum", bufs=3)
                    ctx_ps = psP.tile([128, BQ], F32, tag="ctx", bufs=2)
                    if use_ones:
                        ones_ps = psP.tile([128, BQ], F32, tag="b1", bufs=1)

                    def emit_pv(ck0, gsz):
                        for j in range(gsz):
                            ck = ck0 + j
                            off = max(0, (ck - (nch - 4)) * 128)
                            nc.tensor.matmul(
                                ctx_ps[:, off:BQ],
                                vN[:, ck, h * HD : (h + 1) * HD],
                                expS[:, ck, off:BQ],
                                start=(ck == 0),
                                stop=(ck == nch - 1),
                            )

                    # software-pipelined by one group: PV(g) emitted after
                    # scores(g+1) so the in-order PE queue never stalls
                    prev_group = None
                    ck0 = 0
                    while ck0 < nch:
                        gsz = min(2, nch - ck0)
                        s_ps = psP.tile([128, 2, BQ], F32, tag="s", bufs=2)
                        for j in range(gsz):
                            ck = ck0 + j
                            off = max(0, (ck - (nch - 4)) * 128)
                            nc.tensor.matmul(
                                s_ps[:, j, off:BQ],
                                kT8[hsl, :, ck * 128 : (ck + 1) * 128],
                                qT8[hsl, :, b * BQ + off : (b + 1) * BQ],
                                start=True,
                                stop=True,
                                perf_mode=DR,
                            )
                        if prev_group is not None:
                            emit_pv(*prev_group)
                        # exp only the causally-live columns of the group
                        # (off of the group's FIRST chunk; the mask zeroes the
                        # [off0:off1] sliver of the second chunk)
                        off0 = max(0, (ck0 - (nch - 4)) * 128)
                        nc.scalar.activation(
                            expS[:, ck0 : ck0 + gsz, off0:BQ],
                            s_ps[:, 0:gsz, off0:BQ],
                            Exp,
                            scale=F_EXP,
                        )
                        for j in range(gsz):
                            ck = ck0 + j
                            if ck >= nch - 4:  # diagonal chunk: causal mask
                                off = max(0, (ck - (nch - 4)) * 128)
                                nc.vector.tensor_mul(
                                    expS[:, ck, 0 : off + 128],
                                    expS[:, ck, 0 : off + 128],
                                    cm_t[:, ck - (nch - 4), 0 : off + 128],
                                )
                        if use_ones:
                            for j in range(gsz):
                                ck = ck0 + j
                                off = max(0, (ck - (nch - 4)) * 128)
                                nc.tensor.matmul(
                                    ones_ps[0:1, off:BQ],
                                    ones_t[:, 0:1],
                                    expS[:, ck, off:BQ],
                                    start=(ck == 0),
                                    stop=(ck == nch - 1),
                                )
                        elif not OPTS["no_denom"] and ck0 % 4 == 2:
                            # pair/quad bf16 tree on DVE
                            epair2 = sbAT[0].tile([128, 2, BQ], BF16, tag="epair2", bufs=2)
                            nc.vector.tensor_add(
                                epair2[:],
                                expS[:, ck0 - 2 : ck0, :],
                                expS[:, ck0 : ck0 + 2, :],
                            )
                            if ck0 == 2:
                                nc.vector.tensor_add(
                                    dsum[:], epair2[:, 0, :], epair2[:, 1, :]
                                )
                            else:
                                equad = sbAT[0].tile([128, BQ], BF16, tag="equad", bufs=2)
                                nc.vector.tensor_add(
                                    equad[:], epair2[:, 0, :], epair2[:, 1, :]
                                )
                                nc.vector.tensor_add(dsum[:], dsum[:], equad[:])
                        prev_group = (ck0, gsz)
                        ck0 += gsz
                    emit_pv(*prev_group)
                    if OPTS["no_denom"]:
                        nc.vector.tensor_copy(ctx8h[:, h, qwin], ctx_ps[:])
                        continue
                    bc_t = sbAT[0].tile([128, BQ], F32, tag="bc_t", bufs=3)
                    dsb = sbAT[0].tile([128, BQ], F32, tag="dsb", bufs=3)
                    if use_ones:
                        nc.vector.reciprocal_approx_fast(
                            out=dsb[0:1, :], in_=ones_ps[0:1, :]
                        )
                        nc.gpsimd.partition_broadcast(bc_t[:], dsb[0:1, :])
                    else:
                        nc.gpsimd.partition_all_reduce(
                            dsb[:], dsum[:], channels=128, reduce_op=_ReduceOp.add
                        )
                        nc.vector.reciprocal_approx_fast(out=bc_t[:], in_=dsb[:])
                    t_bf = sbAT[0].tile([128, BQ], BF16, tag="tbf", bufs=3)
                    nc.vector.tensor_mul(t_bf[:], ctx_ps[:], bc_t[:])
                    # hi/lo fp8 split of ctx on the Pool engine
                    nc.gpsimd.tensor_copy(ctx8h[:, h, qwin], t_bf[:])
                    nc.gpsimd.tensor_sub(
                        ctx8l[:, h, qwin], t_bf[:], ctx8h[:, h, qwin]
                    )

            def outproj_block(b):
                # fused out-projection for a q-window (4 row-chunks of 128)
                for j in range(4):
                    lc = b * 4 + j
                    lsl = slice(lc * 128, (lc + 1) * 128)
                    ostage = sbAT[0].tile([128, D], BF16, tag="ostage", bufs=5)
                    for do in range(4):
                        ps = psP.tile(
                            [128, 512], F32, tag=("o" if do % 2 == 0 else "b1"),
                            bufs=1,
                        )
                        osl = slice(do * 512, (do + 1) * 512)
                        for mi, (cx, whl) in enumerate(
                            ((ctx8h, 0), (ctx8l, 0), (ctx8h, 1))
                        ):
                            nc.tensor.matmul(
                                ps[:],
                                cx[:, :, lsl],
                                wo_t[:, whl, :, osl],
                                start=(mi == 0),
                                stop=(mi == 2),
                                perf_mode=DR,
                            )
                        if do % 2 == 0:
                            nc.vector.tensor_scalar_mul(ostage[:, osl], ps[:], F_O)
                        else:
                            nc.scalar.activation(ostage[:, osl], ps[:], Copy, scale=F_O)
                    nc.sync.dma_start(OUT[lsl, :], ostage[:])

            if not OPTS["no_upproj"]:
                for w in range(NB):
                    emit_passA(w)
                for w in range(NB):
                    emit_passB(w)
        if not OPTS["no_attn"]:
            with tc.tile_pool(name="sb_at", bufs=1) as _sbat:
                sbAT[0] = _sbat
                for b in (0, 7, 6, 5, 4, 3, 2, 1):
                    attn_block(b)
                    if not OPTS["no_outproj"]:
                        outproj_block(b)


def _hilo8(a, s):
    h = (a * s).astype(E4)
    lo = (a * s - h.astype(np.float32)).astype(E4)
    return h, lo


def _host_inputs(x, W_dkv, W_dq, W_uk, W_uv, W_uq, W_qr, W_kr, W_out):
    """Build per-core input maps (numpy)."""
    scale = 1.0 / math.sqrt(HD)

    xt = np.ascontiguousarray(x.reshape(L, D).T)  # [D, L] f32

    # rope tables, transposed: ang[l, i] = l * inv_freq[i]
    inv_freq = 1.0 / (ROPE_BASE ** (np.arange(HALF, dtype=np.float64) * 2.0 / ROPE))
    ang = np.arange(L, dtype=np.float64)[:, None] * inv_freq[None, :]  # [L, 32]
    cosT = np.cos(ang).T.astype(np.float32)  # [32, L]
    sinT = np.sin(ang).T.astype(np.float32)
    cc64 = np.concatenate([cosT, cosT], axis=0)        # [64, L]
    ss64 = np.concatenate([-sinT, sinT], axis=0)       # [64, L]
    # F_K (== F_Q) psum->k8 scale folds into the rope tables
    CCm = np.concatenate([cc64, cc64], axis=0) * F_K   # [128, L] (both head slots)
    SSm = np.concatenate([ss64, ss64], axis=0) * F_K

    CM = np.zeros((4, 128, BQ), dtype=np.float32)
    for j in range(4):
        for lk in range(128):
            CM[j, lk, j * 128 + lk :] = 1.0

    # rope half-swap as a column permutation of the rope projection weights
    swap_idx = np.concatenate(
        [np.array([g * 64 + (m + HALF) % 64 for m in range(64)]) for g in range(2)]
    )

    bf = lambda a: np.ascontiguousarray(a).astype(ml_dtypes.bfloat16)
    ONES = np.ones((128, 1), dtype=np.float32)

    wdkv_h, wdkv_l = _hilo8(W_dkv, SWD)
    wdq_h, wdq_l = _hilo8(W_dq, SWD)
    wdkv8 = np.stack([wdkv_h, wdkv_l])
    wdq8 = np.stack([wdq_h, wdq_l])

    in_maps = []
    for r in range(N_CORES):
        heads = [2 * r, 2 * r + 1]
        # column layout: [rope_h0 | rope_h1 | base_h0 | base_h1] (64 each)
        kr = [W_kr[:, hh * ROPE : (hh + 1) * ROPE] for hh in heads]
        kb = [W_uk[:, hh * SPLIT : (hh + 1) * SPLIT] for hh in heads]
        qr = [W_qr[:, hh * ROPE : (hh + 1) * ROPE] * scale for hh in heads]
        qb = [W_uq[:, hh * SPLIT : (hh + 1) * SPLIT] * scale for hh in heads]
        kr_blk = np.concatenate(kr, axis=1)
        qr_blk = np.concatenate(qr, axis=1)
        # [rope | base | rope-halfswapped]
        wk_cols = np.concatenate([kr_blk] + kb + [kr_blk[:, swap_idx]], axis=1)
        wq_cols = np.concatenate([qr_blk] + qb + [qr_blk[:, swap_idx]], axis=1)
        wv_cols = np.concatenate(
            [W_uv[:, hh * HD : (hh + 1) * HD] for hh in heads], axis=1
        )
        wo_rows = np.concatenate(
            [W_out[hh * HD : (hh + 1) * HD, :] for hh in heads], axis=0
        )
        xh, xl = _hilo8(xt[:, r * LLOC : (r + 1) * LLOC], SX)
        wvh, wvl = _hilo8(wv_cols, SWV)
        woh, wol = _hilo8(wo_rows, SWO)
        in_maps.append(
            {
                "xT8": np.stack([xh, xl]),
                "wdkv8": wdkv8,
                "wdq8": wdq8,
                "wk8": (wk_cols * SWK).astype(E4),
                "wq8": (wq_cols * SWQ).astype(E4),
                "wv8": np.stack([wvh, wvl]),
                "wo8": np.stack([woh, wol]),
                "CC": bf(CCm),
                "SS": bf(SSm),
                "CM": bf(CM),
                "ONES": bf(ONES),
            }
        )
    return in_maps


def _get_program(reps=1):
    if reps not in _CACHE:
        _CACHE[reps] = _build_program(reps)
    return _CACHE[reps]


def make_runner(in_maps, reps=1, empty=False):
    """Persistent compiled runner for timing: returns run_chain(M) that executes
    the program M times back-to-back on device (chained via the output buffer so
    executions serialize), returning wall seconds for the chain."""
    import time as _time
    import jax
    from jax.sharding import Mesh, PartitionSpec, NamedSharding
    from jax.experimental.shard_map import shard_map
    import concourse.bass2jax as bass2jax

    nc = _EMPTY_CACHE.setdefault(0, _build_empty()) if empty else _get_program(reps)
    bass2jax.install_neuronx_cc_hook()
    partition_name = nc.partition_id_tensor.name if nc.partition_id_tensor else None
    in_names, out_names, out_avals, zero_outs = [], [], [], []
    for alloc in nc.m.functions[0].allocations:
        if not isinstance(alloc, mybir.MemoryLocationSet):
            continue
        name = alloc.memorylocations[0].name
        if alloc.kind == "ExternalInput":
            if name != partition_name:
                in_names.append(name)
        elif alloc.kind == "ExternalOutput":
            out_names.append(name)
            shape = tuple(alloc.tensor_shape)
            dtype = mybir.dt.np(alloc.dtype)
            out_avals.append(jax.core.ShapedArray(shape, dtype))
            zero_outs.append(np.zeros(shape, dtype))
    n_params = len(in_names)
    in_names_all = in_names + out_names
    if partition_name is not None:
        in_names_all = in_names_all + [partition_name]

    def _body(*args):
        operands = list(args)
        if partition_name is not None:
            operands.append(bass2jax.partition_id_tensor())
        outs = bass2jax._bass_exec_p.bind(
            *operands,
            out_avals=tuple(out_avals),
            in_names=tuple(in_names_all),
            out_names=tuple(out_names),
            lowering_input_output_aliases=(),
            sim_require_finite=True,
            sim_require_nnan=True,
            nc=nc,
        )
        return tuple(outs)

    devices = jax.devices()[:N_CORES]
    mesh = Mesh(np.asarray(devices), ("core",))
    n_outs = len(out_names)
    in_specs = (PartitionSpec("core"),) * (n_params + n_outs)
    out_specs = (PartitionSpec("core"),) * n_outs
    sharded = jax.jit(
        shard_map(_body, mesh=mesh, in_specs=in_specs, out_specs=out_specs, check_rep=False),
        keep_unused=True,
    )
    sh = NamedSharding(mesh, PartitionSpec("core"))
    concat_in = [
        np.concatenate([np.asarray(in_maps[c][nm]) for c in range(N_CORES)], axis=0)
        for nm in in_names
    ]
    concat_zeros = [
        np.zeros((N_CORES * z.shape[0], *z.shape[1:]), z.dtype) for z in zero_outs
    ]
    dev_in = [jax.device_put(a, sh) for a in concat_in]
    dev_zero = [jax.device_put(a, sh) for a in concat_zeros]
    outs = sharded(*dev_in, *dev_zero)
    jax.block_until_ready(outs)  # compile + warm

    def run_chain(M):
        z = list(dev_zero)
        t0 = _time.perf_counter()
        outs = None
        for _ in range(M):
            outs = sharded(*dev_in, *z)
            z = list(outs)
        jax.block_until_ready(outs)
        return _time.perf_counter() - t0

    return run_chain


_EMPTY_CACHE = {}


def _build_empty():
    """Minimal program with the same I/O signature class: one tiny DMA."""
    nc = bacc.Bacc("TRN2", target_bir_lowering=False, debug=False, num_devices=N_CORES)
    xT = nc.dram_tensor("xT", [128, 128], F32, kind="ExternalInput")
    OUT = nc.dram_tensor("OUT", [128, 128], F32, kind="ExternalOutput")
    with tile.TileContext(nc) as tc:
        with tc.tile_pool(name="sb", bufs=1) as sb:
            t = sb.tile([128, 128], F32, tag="t")
            nc.sync.dma_start(t[:], xT[:])
            nc.sync.dma_start(OUT[:], t[:])
    nc.compile()
    return nc


def make_empty_runner():
    in_maps = [{"xT": np.zeros((128, 128), np.float32)} for _ in range(N_CORES)]
    return make_runner(in_maps, empty=True)


def kernel(x, W_dkv, W_dq, W_uk, W_uv, W_uq, W_qr, W_kr, W_out, b_out, reps=1):
    x = np.asarray(x, dtype=np.float32)
    in_maps = _host_inputs(
        x,
        np.asarray(W_dkv, np.float32), np.asarray(W_dq, np.float32),
        np.asarray(W_uk, np.float32), np.asarray(W_uv, np.float32),
        np.asarray(W_uq, np.float32), np.asarray(W_qr, np.float32),
        np.asarray(W_kr, np.float32), np.asarray(W_out, np.float32),
    )
    nc = _get_program(reps)
    res = run_bass_kernel_spmd(nc, in_maps, core_ids=list(range(N_CORES)), trace=False)
    out = np.zeros((L, D), dtype=np.float32)
    for r in range(N_CORES):
        out += res.results[r]["OUT"].astype(np.float32)
    out += np.asarray(b_out, np.float32)[None, :]
    return out.reshape(1, L, D)
